# revision 14
# baseline (speedup 1.0000x reference)
"""Trainium2 Bass kernel for nn_BiDGNBlock (moe_routing).

Strategy: data-parallel over batch across 8 NeuronCores (no collectives).
Each core computes one batch element end-to-end:
  - BiMultiHeadAttention + layernorms + residuals in exact fp32 (the cosine
    router's top-2 picks have sim-gaps down to 4.4e-5, so this path must
    match the fp32 reference closely).
  - CosineRouter: cosine sims via fp32 matmuls, top-2 indices via the DVE
    max/max_index top-8 instruction. The gate scalar (softmax over the top-2
    re-normalized, summed) is exactly 1.0, so only the indices matter.
  - Per-channel experts: the full expert table We (64x256x256) is held in
    SBUF as fp16; the routed sum  moe[c] = sum_k out[c] @ We[topi[c,k]].T
    is computed as  sum_e (mask_e * A).T-matmuls accumulated in PSUM, where
    mask_e[c] = [expert e in topi[c]] is built on-device from the indices.
  - Final layernorm + residual, fp32.
"""

import sys
import numpy as np

sys.path.insert(0, "/opt/trn_rl_repo")

N_CORES = 8
B, C, T = 8, 64, 256
EXP = 32
KT = T // 128  # 2 k-tiles over the feature dim

_CACHE: dict = {}

# fp32 blob layouts: (name, partitions, shape). cols = prod(shape[1:]).
BLOB_A_SPEC = [
    ("xtl", 128, (128, KT, C)), ("xtr", 128, (128, KT, C)),
    ("wqt", 128, (128, KT, T)), ("wkt", 128, (128, KT, T)),
    ("wvt", 128, (128, KT, T)),
    ("bqp", 128, (128, KT)), ("bkp", 128, (128, KT)),
]
BLOB_B_SPEC = [
    ("wpt", 128, (128, KT, T)), ("wrt", 128, (128, 2 * KT, EXP)),
    ("ident", 128, (128, 128)), ("sel", 2, (2, 2, 128)),
    ("xl", 64, (64, T)), ("xr", 64, (64, T)),
    ("bv", 64, (64, T)), ("bp", 64, (64, T)),
    ("agl", 64, (64, T)), ("abl", 64, (64, T)),
    ("agr", 64, (64, T)), ("abr", 64, (64, T)),
    ("mgl", 64, (64, T)), ("mbl", 64, (64, T)),
    ("mgr", 64, (64, T)), ("mbr", 64, (64, T)),
    ("brp", 32, (32, 1)), ("cent", 32, (32, C)), ("eiota", 64, (64, 1)),
]


def _blob_layout():
    off = {}
    na = 0
    for name, parts, shape in BLOB_A_SPEC:
        cols = int(np.prod(shape[1:]))
        off[name] = (na, parts, shape)
        na += cols
    nb = 0
    for name, parts, shape in BLOB_B_SPEC:
        cols = int(np.prod(shape[1:]))
        off[name] = (nb, parts, shape)
        nb += cols
    return off, na, nb


BLOB_OFF, NA_COLS, NB_COLS = _blob_layout()


def _build():
    import concourse.bass as bass
    import concourse.mybir as mybir
    import concourse.tile as tile
    from concourse import bacc
    from contextlib import ExitStack

    dt = mybir.dt
    f32, f16 = dt.float32, dt.float16
    AF = mybir.ActivationFunctionType
    OP = mybir.AluOpType

    nc = bacc.Bacc("TRN2", target_bir_lowering=False, debug=False,
                   num_devices=N_CORES)

    def inp(name, shape, d=f32):
        return nc.dram_tensor(name, list(shape), d, kind="ExternalInput")

    vec_names = ["bv", "bp", "agl", "abl", "agr", "abr",
                 "mgl", "mbl", "mgr", "mbr"]
    # fp32 constants+inputs packed host-side into two blobs (one DMA each).
    # BLOB_SPEC: (name, partitions, shape) -- order shared with _prep_in_maps.
    blobA_d = inp("blobA", (128, NA_COLS))
    blobB_d = inp("blobB", (128, NB_COLS))
    weh_d = inp("weh", (128, C, KT, T), f16)  # We[e].T tiled [p, e, kt, u] fp16
    beh_d = inp("beh", (C, T), f16)           # be natural fp16

    ol2_d = nc.dram_tensor("ol2", [C, T], f32, kind="ExternalOutput")
    or2_d = nc.dram_tensor("or2", [C, T], f32, kind="ExternalOutput")

    with tile.TileContext(nc) as tc, ExitStack() as ctx:
        cst = ctx.enter_context(tc.tile_pool(name="cst", bufs=1))
        wk = ctx.enter_context(tc.tile_pool(name="wk", bufs=2))
        sm = ctx.enter_context(tc.tile_pool(name="sm", bufs=2))
        asc_p = ctx.enter_context(tc.tile_pool(name="asc", bufs=4))
        msk_p = ctx.enter_context(tc.tile_pool(name="msk", bufs=4))
        ps = ctx.enter_context(tc.tile_pool(name="ps", bufs=3, space="PSUM"))
        ps_moe_p = ctx.enter_context(tc.tile_pool(name="psmoe", bufs=1, space="PSUM"))

        def load(pool, d_tensor, shape, dty=f32, tag=None):
            t = pool.tile(list(shape), dty, tag=tag or d_tensor.name)
            nc.sync.dma_start(out=t, in_=d_tensor.ap())
            return t

        # ---- loads: blobA (attention-critical) first, then blobB, weh ----
        blobA = cst.tile([128, NA_COLS], f32, tag="blobA")
        nc.sync.dma_start(out=blobA, in_=blobA_d.ap())
        blobB = cst.tile([128, NB_COLS], f32, tag="blobB")
        nc.sync.dma_start(out=blobB, in_=blobB_d.ap())
        we_sb = cst.tile([128, C, KT, T], f16, tag="weh")
        wea = weh_d.ap()
        for ch in range(4):  # SWDGE queue, parallel to the sync loads
            nc.gpsimd.dma_start(out=we_sb[:, ch * 16:(ch + 1) * 16],
                                in_=wea[:, ch * 16:(ch + 1) * 16])
        beh = cst.tile([C, T], f16, tag="beh")
        nc.sync.dma_start(out=beh, in_=beh_d.ap())

        def bview(blob, name):
            off, parts, shape = BLOB_OFF[name]
            cols = 1
            for s in shape[1:]:
                cols *= s
            v = blob[0:parts, off:off + cols]
            if len(shape) == 3:
                v = v.rearrange("p (a b) -> p a b", a=shape[1])
            return v

        xtl = bview(blobA, "xtl")
        xtr = bview(blobA, "xtr")
        wqt = bview(blobA, "wqt")
        wkt = bview(blobA, "wkt")
        wvt = bview(blobA, "wvt")
        bqp = bview(blobA, "bqp")
        bkp = bview(blobA, "bkp")
        wpt = bview(blobB, "wpt")
        wrt = bview(blobB, "wrt")
        ident = bview(blobB, "ident")
        sel = bview(blobB, "sel")
        xl_sb = bview(blobB, "xl")
        xr_sb = bview(blobB, "xr")
        vec = {n: bview(blobB, n) for n in vec_names}
        brp = bview(blobB, "brp")
        cent = bview(blobB, "cent")
        eiota = bview(blobB, "eiota")

        eps_t = cst.tile([C, 1], f32, tag="eps")
        nc.vector.memset(eps_t, 1e-5)

        # PE warm-up from a memset tile (no DMA dependency): HAM reaches
        # K=8/8 during the input DMA window. Also preload the ACT tables.
        warm_p = ctx.enter_context(tc.tile_pool(name="warm", bufs=1, space="PSUM"))
        wsrc = cst.tile([128, 512], f16, tag="wsrc")
        nc.vector.memset(wsrc, 0.5)
        pw = warm_p.tile([128, 512], f32, tag="warm")
        for wi in range(14):
            nc.tensor.matmul(pw, wsrc[:, 0:128], wsrc,
                             start=True, stop=True, skip_group_check=True)
        wact = cst.tile([1, 32], f32, tag="wact")
        nc.vector.memset(wact, 1.0)
        nc.scalar.activation(out=wact, in_=wact, func=AF.Exp)
        nc.scalar.activation(out=wact, in_=wact, func=AF.Sqrt)

        # ---- attention: q.T, k.T ----
        qt = wk.tile([128, KT, C], f32, tag="qt")
        ktl = wk.tile([128, KT, C], f32, tag="ktl")
        for (src, w, bias, dst) in [(xtl, wqt, bqp, qt), (xtr, wkt, bkp, ktl)]:
            for ut in range(KT):
                p = ps.tile([128, C], f32, tag="ps")
                for kt in range(KT):
                    nc.tensor.matmul(p, w[:, kt, ut * 128:(ut + 1) * 128],
                                     src[:, kt], start=(kt == 0), stop=(kt == KT - 1))
                nc.vector.tensor_scalar(out=dst[:, ut], in0=p,
                                        scalar1=bias[:, ut:ut + 1], scalar2=None,
                                        op0=OP.add)

        # ---- v = (x_l - x_r) @ Wv.T + bv  (natural layout [c, u]) ----
        xdt = wk.tile([128, KT, C], f32, tag="xdt")
        nc.vector.tensor_sub(xdt, xtl, xtr)
        pv = ps.tile([C, T], f32, tag="ps")
        for kt in range(KT):
            nc.tensor.matmul(pv, xdt[:, kt], wvt[:, kt],
                             start=(kt == 0), stop=(kt == KT - 1))
        v_sb = wk.tile([C, T], f32, tag="v")
        nc.vector.tensor_tensor(out=v_sb, in0=pv, in1=vec["bv"], op=OP.add)

        # ---- energy + softmax ----
        pe_ = ps.tile([C, C], f32, tag="ps")
        for ut in range(KT):
            nc.tensor.matmul(pe_, qt[:, ut], ktl[:, ut],
                             start=(ut == 0), stop=(ut == KT - 1))
        rowmax = sm.tile([C, 1], f32, tag="rowmax")
        nc.vector.tensor_reduce(rowmax, pe_, axis=mybir.AxisListType.X, op=OP.max)
        attn = wk.tile([C, C], f32, tag="attn")
        nc.vector.tensor_scalar(out=attn, in0=pe_, scalar1=rowmax, scalar2=1.0 / 16.0,
                                op0=OP.subtract, op1=OP.mult)
        nc.scalar.activation(out=attn, in_=attn, func=AF.Exp)
        rowsum = sm.tile([C, 1], f32, tag="rowsum")
        nc.vector.tensor_reduce(rowsum, attn, axis=mybir.AxisListType.X, op=OP.add)
        nc.vector.reciprocal(rowsum, rowsum)
        nc.vector.tensor_scalar_mul(attn, attn, rowsum)

        # ---- attn.T ----
        pat = ps.tile([C, C], f32, tag="ps")
        nc.tensor.transpose(pat, attn, ident[0:C, 0:C])
        attnT = wk.tile([C, C], f32, tag="attnT")
        nc.vector.tensor_copy(attnT, pat)

        # ---- out_l.T / out_r.T  [u, c] ----
        oLT = wk.tile([128, KT, C], f32, tag="oLT")
        oRT = wk.tile([128, KT, C], f32, tag="oRT")
        for ut in range(KT):
            pl = ps.tile([128, C], f32, tag="ps")
            nc.tensor.matmul(pl, v_sb[:, ut * 128:(ut + 1) * 128], attnT,
                             start=True, stop=True)
            nc.vector.tensor_copy(oLT[:, ut], pl)
            pr = ps.tile([128, C], f32, tag="ps")
            nc.tensor.matmul(pr, v_sb[:, ut * 128:(ut + 1) * 128], attn,
                             start=True, stop=True)
            nc.vector.tensor_copy(oRT[:, ut], pr)

        # ---- proj + LN + residual -> OUT_L / OUT_R (natural [c, u]) ----
        def ln_block(src_ps, bias_t, gamma, beta, resid, out_tile, stag):
            if bias_t is not None:
                nc.vector.tensor_tensor(out=out_tile, in0=src_ps, in1=bias_t,
                                        op=OP.add)
            else:
                nc.vector.tensor_copy(out_tile, src_ps)
            stats = sm.tile([C, 6], f32, tag="stats" + stag)
            nc.vector.bn_stats(out=stats, in_=out_tile)
            mv = sm.tile([C, 2], f32, tag="mv" + stag)
            nc.vector.bn_aggr(out=mv, in_=stats)
            rstd = sm.tile([C, 1], f32, tag="rstd" + stag)
            nc.scalar.activation(out=rstd, in_=mv[:, 1:2], func=AF.Sqrt,
                                 bias=eps_t)
            nc.vector.reciprocal(rstd, rstd)
            nc.vector.tensor_scalar(out=out_tile, in0=out_tile,
                                    scalar1=mv[:, 0:1], scalar2=rstd,
                                    op0=OP.subtract, op1=OP.mult)
            nc.vector.tensor_tensor(out=out_tile, in0=out_tile, in1=gamma,
                                    op=OP.mult)
            nc.vector.tensor_tensor(out=out_tile, in0=out_tile, in1=beta,
                                    op=OP.add)
            nc.vector.tensor_tensor(out=out_tile, in0=out_tile, in1=resid,
                                    op=OP.add)

        OUT_L = wk.tile([C, T], f32, tag="OUTL")
        OUT_R = wk.tile([C, T], f32, tag="OUTR")
        for (oT, g, bt, resid, out_t) in [
                (oLT, vec["agl"], vec["abl"], xl_sb, OUT_L),
                (oRT, vec["agr"], vec["abr"], xr_sb, OUT_R)]:
            pp = ps.tile([C, T], f32, tag="ps")
            for ut in range(KT):
                nc.tensor.matmul(pp, oT[:, ut], wpt[:, ut],
                                 start=(ut == 0), stop=(ut == KT - 1))
            ln_block(pp, vec["bp"], g, bt, resid, out_t, "1")

        # ---- transposes of OUT_L/OUT_R -> [u(128), kt, c] tiles ----
        oLT2 = wk.tile([128, KT, C], f32, tag="oLT2")
        oRT2 = wk.tile([128, KT, C], f32, tag="oRT2")
        for (src, dst) in [(OUT_L, oLT2), (OUT_R, oRT2)]:
            for ut in range(KT):
                pt = ps.tile([128, C], f32, tag="ps")
                nc.tensor.transpose(pt, src[:, ut * 128:(ut + 1) * 128],
                                    ident[0:C, 0:C])
                nc.vector.tensor_copy(dst[:, ut], pt)

        # ---- router ----
        rtiles = [(oLT2, 0), (oLT2, 1), (oRT2, 0), (oRT2, 1)]
        # xp.T [d, c] (for the sim matmul's contraction over d)
        pxp = ps.tile([EXP, C], f32, tag="ps")
        for j, (tl, kt) in enumerate(rtiles):
            nc.tensor.matmul(pxp, wrt[:, j], tl[:, kt],
                             start=(j == 0), stop=(j == 3))
        xpT = wk.tile([EXP, C], f32, tag="xpT")
        nc.vector.tensor_scalar(out=xpT, in0=pxp, scalar1=brp, scalar2=None,
                                op0=OP.add)
        # xp natural [c, d] via PE transpose of xpT (for the row norms)
        pxn = ps.tile([C, EXP], f32, tag="ps")
        nc.tensor.transpose(pxn, xpT, ident[0:EXP, 0:EXP])
        xpn = sm.tile([C, EXP], f32, tag="xpn")
        nc.vector.tensor_copy(xpn, pxn)

        sq = sm.tile([C, EXP], f32, tag="sq")
        nc.vector.tensor_mul(sq, xpn, xpn)
        ssum = sm.tile([C, 1], f32, tag="ssum")
        nc.vector.tensor_reduce(ssum, sq, axis=mybir.AxisListType.X, op=OP.add)
        nc.scalar.activation(out=ssum, in_=ssum, func=AF.Sqrt)
        nc.vector.tensor_scalar_max(ssum, ssum, 1e-12)
        nc.vector.reciprocal(ssum, ssum)

        psim = ps.tile([C, C], f32, tag="ps")
        nc.tensor.matmul(psim, xpT, cent, start=True, stop=True)
        sim_sb = wk.tile([C, C], f32, tag="sim")
        nc.vector.tensor_scalar_mul(sim_sb, psim, ssum)

        mx8 = sm.tile([C, 8], f32, tag="mx8")
        nc.vector.max(out=mx8, in_=sim_sb)
        idx8 = sm.tile([C, 8], mybir.dt.uint32, tag="idx8")
        nc.vector.max_index(out=idx8, in_max=mx8, in_values=sim_sb)
        topif = sm.tile([C, 2], f32, tag="topif")
        nc.vector.tensor_copy(topif, idx8[:, 0:2])

        # ---- replicate topi rows across all 128 partitions via PE ----
        ptt = ps.tile([2, C], f32, tag="ps")
        nc.tensor.transpose(ptt, topif, ident[0:C, 0:C])
        ttT = sm.tile([2, C], f32, tag="ttT")
        nc.vector.tensor_copy(ttT, ptt)
        ttrep_ps = []
        for k in range(2):
            pr = ps.tile([128, C], f32, tag="ps")
            nc.tensor.matmul(pr, sel[:, k], ttT, start=True, stop=True)
            ttrep_ps.append(pr)

        # R.T[e, c] for the bias matmul (fp16)
        RT = wk.tile([C, C], f32, tag="RT")
        RT1 = sm.tile([C, C], f32, tag="RT1")
        nc.vector.tensor_scalar(out=RT, in0=ttrep_ps[0][0:C], scalar1=eiota,
                                scalar2=None, op0=OP.is_equal)
        nc.vector.tensor_scalar(out=RT1, in0=ttrep_ps[1][0:C], scalar1=eiota,
                                scalar2=None, op0=OP.is_equal)
        nc.vector.tensor_add(RT, RT, RT1)
        RTh = wk.tile([C, C], f16, tag="RTh")
        nc.vector.tensor_copy(RTh, RT)

        # ---- fp16 copy of transposed activations [p, kt, side, c] ----
        oAll = wk.tile([128, KT, 2, C], f16, tag="oAll")
        for kt in range(KT):
            nc.vector.tensor_copy(oAll[:, kt, 0], oLT2[:, kt])
            nc.vector.tensor_copy(oAll[:, kt, 1], oRT2[:, kt])

        # ---- ttrep rows as fp16 for the per-expert masks ----
        tt0r = wk.tile([128, C], f16, tag="tt0r")
        tt1r = wk.tile([128, C], f16, tag="tt1r")
        nc.vector.tensor_copy(tt0r, ttrep_ps[0])
        nc.vector.tensor_copy(tt1r, ttrep_ps[1])

        # ---- expert stage: per-expert mask + one fused multiply ----
        ps_moe = ps_moe_p.tile([128, T], f32, tag="psmoe")
        for e in range(C):
            m0 = msk_p.tile([128, C], f16, tag="m0")
            m1 = msk_p.tile([128, C], f16, tag="m1")
            nc.vector.tensor_scalar(out=m0, in0=tt0r, scalar1=float(e),
                                    scalar2=None, op0=OP.is_equal)
            nc.vector.tensor_scalar(out=m1, in0=tt1r, scalar1=float(e),
                                    scalar2=None, op0=OP.is_equal)
            nc.vector.tensor_add(m0, m0, m1)
            asc = asc_p.tile([128, KT, 2, C], f16, tag="asc")
            m0b = bass.AP(tensor=m0.tensor, offset=m0.offset,
                          ap=[list(m0.ap[0]), [0, KT], [0, 2], list(m0.ap[1])])
            nc.vector.tensor_tensor(out=asc, in0=oAll, in1=m0b, op=OP.mult)
            for kt in range(KT):
                nc.tensor.matmul(ps_moe, asc[:, kt], we_sb[:, e, kt],
                                 start=(e == 0 and kt == 0), stop=False,
                                 skip_group_check=True)
                nc.tensor.matmul(ps_moe[0:C], RTh, beh, start=False, stop=False,
                         skip_group_check=True)
        nc.tensor.matmul(ps_moe[C:128], RTh, beh, start=False, stop=True,
                         skip_group_check=True)

        # ---- final LN + residual ----
        ol2 = wk.tile([C, T], f32, tag="ol2")
        or2 = wk.tile([C, T], f32, tag="or2")
        ln_block(ps_moe[0:C], None, vec["mgl"], vec["mbl"], OUT_L, ol2, "2")
        ln_block(ps_moe[C:128], None, vec["mgr"], vec["mbr"], OUT_R, or2, "2")
        nc.sync.dma_start(out=ol2_d.ap(), in_=ol2)
        nc.sync.dma_start(out=or2_d.ap(), in_=or2)

    nc.compile()
    return nc


def _tile_t(w):
    # (T_in, N) -> [128, T_in//128, N] partition-tiled
    t_in, n = w.shape
    return np.ascontiguousarray(w.reshape(t_in // 128, 128, n).transpose(1, 0, 2))


def _prep_in_maps(inputs):
    f = np.float32
    x_l, x_r = inputs["x_l"], inputs["x_r"]

    def rep(name):
        return np.repeat(np.asarray(inputs[name], f).reshape(1, T), C, axis=0)

    cen = np.asarray(inputs["centers"], f)
    cenn = cen / np.maximum(np.linalg.norm(cen, axis=-1, keepdims=True), 1e-12)
    sel = np.zeros((2, 2, 128), f)
    sel[0, 0, :] = 1.0
    sel[1, 1, :] = 1.0
    arrs = {
        "wqt": _tile_t(np.asarray(inputs["Wq"], f).T),
        "wkt": _tile_t(np.asarray(inputs["Wk"], f).T),
        "wvt": _tile_t(np.asarray(inputs["Wv"], f).T),
        "wpt": _tile_t(np.asarray(inputs["Wp"], f).T),
        "bqp": np.asarray(inputs["bq"], f).reshape(KT, 128).T,
        "bkp": np.asarray(inputs["bk"], f).reshape(KT, 128).T,
        "wrt": _tile_t(np.asarray(inputs["Wr"], f).T),
        "brp": np.asarray(inputs["br"], f).reshape(EXP, 1),
        "cent": np.ascontiguousarray(cenn.T),
        "ident": np.eye(128, dtype=f),
        "eiota": np.arange(C, dtype=f).reshape(C, 1),
        "sel": sel,
        "bv": rep("bv"), "bp": rep("bp"),
        "agl": rep("ag_l"), "abl": rep("ab_l"),
        "agr": rep("ag_r"), "abr": rep("ab_r"),
        "mgl": rep("mg_l"), "mbl": rep("mb_l"),
        "mgr": rep("mg_r"), "mbr": rep("mb_r"),
        "xl": np.zeros((C, T), f), "xr": np.zeros((C, T), f),
    }
    We = np.asarray(inputs["We"], f)
    WeTh = np.ascontiguousarray(
        We.transpose(0, 2, 1).reshape(C, KT, 128, T).transpose(2, 0, 1, 3)
    ).astype(np.float16)
    beh = np.asarray(inputs["be"], f).astype(np.float16)

    def pack(spec, ncols, extra):
        blob = np.zeros((128, ncols), f)
        for name, parts, shape in spec:
            off, _, _ = BLOB_OFF[name]
            cols = int(np.prod(shape[1:]))
            a = extra[name] if name in extra else arrs[name]
            blob[0:parts, off:off + cols] = np.asarray(a, f).reshape(parts, cols)
        return blob

    blobB = pack(BLOB_B_SPEC, NB_COLS, {})
    in_maps = []
    for b in range(N_CORES):
        xtl = _tile_t(np.ascontiguousarray(np.asarray(x_l[b], f).T))
        xtr = _tile_t(np.ascontiguousarray(np.asarray(x_r[b], f).T))
        blobA = pack(BLOB_A_SPEC, NA_COLS, {"xtl": xtl, "xtr": xtr})
        bB = blobB.copy()
        o, p, sh = BLOB_OFF["xl"]
        bB[0:p, o:o + T] = np.asarray(x_l[b], f)
        o, p, sh = BLOB_OFF["xr"]
        bB[0:p, o:o + T] = np.asarray(x_r[b], f)
        in_maps.append({"blobA": blobA, "blobB": bB, "weh": WeTh, "beh": beh})
    return in_maps


def kernel(**inputs) -> np.ndarray:
    from concourse.bass_utils import run_bass_kernel_spmd

    if "nc" not in _CACHE:
        _CACHE["nc"] = _build()
    nc = _CACHE["nc"]
    in_maps = _prep_in_maps(inputs)
    res = run_bass_kernel_spmd(nc, in_maps, list(range(N_CORES)))
    _CACHE["exec_time_ns"] = res.exec_time_ns
    out_l2 = np.stack([res.results[b]["ol2"] for b in range(N_CORES)])
    out_r2 = np.stack([res.results[b]["or2"] for b in range(N_CORES)])
    return np.stack([out_l2, out_r2]).astype(np.float32)


# revision 15
# speedup vs baseline: 1.5876x; 1.5876x over previous
"""Trainium2 Bass kernel for nn_BiDGNBlock (moe_routing).

Strategy: data-parallel over batch across 8 NeuronCores (no collectives).
Each core computes one batch element end-to-end:
  - BiMultiHeadAttention + layernorms + residuals in exact fp32 (the cosine
    router's top-2 picks have sim-gaps down to 4.4e-5, so this path must
    match the fp32 reference closely).
  - CosineRouter: cosine sims via fp32 matmuls, top-2 indices via the DVE
    max/max_index top-8 instruction. The gate scalar (softmax over the top-2
    re-normalized, summed) is exactly 1.0, so only the indices matter.
  - Per-channel experts: the full expert table We (64x256x256) is held in
    SBUF as fp16; the routed sum  moe[c] = sum_k out[c] @ We[topi[c,k]].T
    is computed as  sum_e (mask_e * A).T-matmuls accumulated in PSUM, where
    mask_e[c] = [expert e in topi[c]] is built on-device from the indices.
  - Final layernorm + residual, fp32.
"""

import sys
import numpy as np

sys.path.insert(0, "/opt/trn_rl_repo")

N_CORES = 8
B, C, T = 8, 64, 256
EXP = 32
KT = T // 128  # 2 k-tiles over the feature dim

_CACHE: dict = {}

# fp32 blob layouts: (name, partitions, shape). cols = prod(shape[1:]).
BLOB_A_SPEC = [
    ("xtl", 128, (128, KT, C)), ("xtr", 128, (128, KT, C)),
    ("wqt", 128, (128, KT, T)), ("wkt", 128, (128, KT, T)),
    ("wvt", 128, (128, KT, T)),
    ("bqp", 128, (128, KT)), ("bkp", 128, (128, KT)),
]
BLOB_B_SPEC = [
    ("wpt", 128, (128, KT, T)), ("wrt", 128, (128, 2 * KT, EXP)),
    ("ident", 128, (128, 128)), ("sel", 2, (2, 2, 128)),
    ("xl", 64, (64, T)), ("xr", 64, (64, T)),
    ("bv", 64, (64, T)), ("bp", 64, (64, T)),
    ("agl", 64, (64, T)), ("abl", 64, (64, T)),
    ("agr", 64, (64, T)), ("abr", 64, (64, T)),
    ("mgl", 64, (64, T)), ("mbl", 64, (64, T)),
    ("mgr", 64, (64, T)), ("mbr", 64, (64, T)),
    ("brp", 32, (32, 1)), ("cent", 32, (32, C)), ("eiota", 64, (64, 1)),
]


def _blob_layout():
    off = {}
    na = 0
    for name, parts, shape in BLOB_A_SPEC:
        cols = int(np.prod(shape[1:]))
        off[name] = (na, parts, shape)
        na += cols
    nb = 0
    for name, parts, shape in BLOB_B_SPEC:
        cols = int(np.prod(shape[1:]))
        off[name] = (nb, parts, shape)
        nb += cols
    return off, na, nb


BLOB_OFF, NA_COLS, NB_COLS = _blob_layout()


def _build():
    import concourse.bass as bass
    import concourse.mybir as mybir
    import concourse.tile as tile
    from concourse import bacc
    from contextlib import ExitStack

    dt = mybir.dt
    f32, f16 = dt.float32, dt.float16
    AF = mybir.ActivationFunctionType
    OP = mybir.AluOpType

    nc = bacc.Bacc("TRN2", target_bir_lowering=False, debug=False,
                   num_devices=N_CORES)

    def inp(name, shape, d=f32):
        return nc.dram_tensor(name, list(shape), d, kind="ExternalInput")

    vec_names = ["bv", "bp", "agl", "abl", "agr", "abr",
                 "mgl", "mbl", "mgr", "mbr"]
    # fp32 constants+inputs packed host-side into two blobs (one DMA each).
    # BLOB_SPEC: (name, partitions, shape) -- order shared with _prep_in_maps.
    blobA_d = inp("blobA", (128, NA_COLS))
    blobB_d = inp("blobB", (128, NB_COLS))
    weh_d = inp("weh", (128, C, KT, T), f16)  # We[e].T tiled [p, e, kt, u] fp16
    beh_d = inp("beh", (C, T), f16)           # be natural fp16

    ol2_d = nc.dram_tensor("ol2", [C, T], f32, kind="ExternalOutput")
    or2_d = nc.dram_tensor("or2", [C, T], f32, kind="ExternalOutput")

    with tile.TileContext(nc) as tc, ExitStack() as ctx:
        cst = ctx.enter_context(tc.tile_pool(name="cst", bufs=1))
        wk = ctx.enter_context(tc.tile_pool(name="wk", bufs=2))
        sm = ctx.enter_context(tc.tile_pool(name="sm", bufs=2))
        asc_p = ctx.enter_context(tc.tile_pool(name="asc", bufs=4))
        msk_p = ctx.enter_context(tc.tile_pool(name="msk", bufs=4))
        ps = ctx.enter_context(tc.tile_pool(name="ps", bufs=3, space="PSUM"))
        ps_moe_p = ctx.enter_context(tc.tile_pool(name="psmoe", bufs=1, space="PSUM"))

        def load(pool, d_tensor, shape, dty=f32, tag=None):
            t = pool.tile(list(shape), dty, tag=tag or d_tensor.name)
            nc.sync.dma_start(out=t, in_=d_tensor.ap())
            return t

        # ---- loads: blobA (attention-critical) first, then blobB, weh ----
        blobA = cst.tile([128, NA_COLS], f32, tag="blobA")
        nc.sync.dma_start(out=blobA, in_=blobA_d.ap())
        blobB = cst.tile([128, NB_COLS], f32, tag="blobB")
        nc.sync.dma_start(out=blobB, in_=blobB_d.ap())
        we_sb = cst.tile([128, C, KT, T], f16, tag="weh")
        wea = weh_d.ap()
        for ch in range(4):  # SWDGE queue, parallel to the sync loads
            nc.gpsimd.dma_start(out=we_sb[:, ch * 16:(ch + 1) * 16],
                                in_=wea[:, ch * 16:(ch + 1) * 16])
        beh = cst.tile([C, T], f16, tag="beh")
        nc.sync.dma_start(out=beh, in_=beh_d.ap())

        def bview(blob, name):
            off, parts, shape = BLOB_OFF[name]
            cols = 1
            for s in shape[1:]:
                cols *= s
            v = blob[0:parts, off:off + cols]
            if len(shape) == 3:
                v = v.rearrange("p (a b) -> p a b", a=shape[1])
            return v

        xtl = bview(blobA, "xtl")
        xtr = bview(blobA, "xtr")
        wqt = bview(blobA, "wqt")
        wkt = bview(blobA, "wkt")
        wvt = bview(blobA, "wvt")
        bqp = bview(blobA, "bqp")
        bkp = bview(blobA, "bkp")
        wpt = bview(blobB, "wpt")
        wrt = bview(blobB, "wrt")
        ident = bview(blobB, "ident")
        sel = bview(blobB, "sel")
        xl_sb = bview(blobB, "xl")
        xr_sb = bview(blobB, "xr")
        vec = {n: bview(blobB, n) for n in vec_names}
        brp = bview(blobB, "brp")
        cent = bview(blobB, "cent")
        eiota = bview(blobB, "eiota")

        eps_t = cst.tile([C, 1], f32, tag="eps")
        nc.vector.memset(eps_t, 1e-5)

        # PE warm-up from a memset tile (no DMA dependency): HAM reaches
        # K=8/8 during the input DMA window. Also preload the ACT tables.
        warm_p = ctx.enter_context(tc.tile_pool(name="warm", bufs=1, space="PSUM"))
        wsrc = cst.tile([128, 512], f16, tag="wsrc")
        nc.vector.memset(wsrc, 0.5)
        pw = warm_p.tile([128, 512], f32, tag="warm")
        for wi in range(14):
            nc.tensor.matmul(pw, wsrc[:, 0:128], wsrc,
                             start=True, stop=True, skip_group_check=True)
        wact = cst.tile([1, 32], f32, tag="wact")
        nc.vector.memset(wact, 1.0)
        nc.scalar.activation(out=wact, in_=wact, func=AF.Exp)
        nc.scalar.activation(out=wact, in_=wact, func=AF.Sqrt)

        # ---- attention: q.T, k.T ----
        qt = wk.tile([128, KT, C], f32, tag="qt")
        ktl = wk.tile([128, KT, C], f32, tag="ktl")
        for (src, w, bias, dst) in [(xtl, wqt, bqp, qt), (xtr, wkt, bkp, ktl)]:
            for ut in range(KT):
                p = ps.tile([128, C], f32, tag="ps")
                for kt in range(KT):
                    nc.tensor.matmul(p, w[:, kt, ut * 128:(ut + 1) * 128],
                                     src[:, kt], start=(kt == 0), stop=(kt == KT - 1))
                nc.vector.tensor_scalar(out=dst[:, ut], in0=p,
                                        scalar1=bias[:, ut:ut + 1], scalar2=None,
                                        op0=OP.add)

        # ---- v = (x_l - x_r) @ Wv.T + bv  (natural layout [c, u]) ----
        xdt = wk.tile([128, KT, C], f32, tag="xdt")
        nc.vector.tensor_sub(xdt, xtl, xtr)
        pv = ps.tile([C, T], f32, tag="ps")
        for kt in range(KT):
            nc.tensor.matmul(pv, xdt[:, kt], wvt[:, kt],
                             start=(kt == 0), stop=(kt == KT - 1))
        v_sb = wk.tile([C, T], f32, tag="v")
        nc.vector.tensor_tensor(out=v_sb, in0=pv, in1=vec["bv"], op=OP.add)

        # ---- energy + softmax ----
        pe_ = ps.tile([C, C], f32, tag="ps")
        for ut in range(KT):
            nc.tensor.matmul(pe_, qt[:, ut], ktl[:, ut],
                             start=(ut == 0), stop=(ut == KT - 1))
        rowmax = sm.tile([C, 1], f32, tag="rowmax")
        nc.vector.tensor_reduce(rowmax, pe_, axis=mybir.AxisListType.X, op=OP.max)
        attn = wk.tile([C, C], f32, tag="attn")
        nc.vector.tensor_scalar(out=attn, in0=pe_, scalar1=rowmax, scalar2=1.0 / 16.0,
                                op0=OP.subtract, op1=OP.mult)
        nc.scalar.activation(out=attn, in_=attn, func=AF.Exp)
        rowsum = sm.tile([C, 1], f32, tag="rowsum")
        nc.vector.tensor_reduce(rowsum, attn, axis=mybir.AxisListType.X, op=OP.add)
        nc.vector.reciprocal(rowsum, rowsum)
        nc.vector.tensor_scalar_mul(attn, attn, rowsum)

        # ---- attn.T ----
        pat = ps.tile([C, C], f32, tag="ps")
        nc.tensor.transpose(pat, attn, ident[0:C, 0:C])
        attnT = wk.tile([C, C], f32, tag="attnT")
        nc.vector.tensor_copy(attnT, pat)

        # ---- out_l.T / out_r.T  [u, c] ----
        oLT = wk.tile([128, KT, C], f32, tag="oLT")
        oRT = wk.tile([128, KT, C], f32, tag="oRT")
        for ut in range(KT):
            pl = ps.tile([128, C], f32, tag="ps")
            nc.tensor.matmul(pl, v_sb[:, ut * 128:(ut + 1) * 128], attnT,
                             start=True, stop=True)
            nc.vector.tensor_copy(oLT[:, ut], pl)
            pr = ps.tile([128, C], f32, tag="ps")
            nc.tensor.matmul(pr, v_sb[:, ut * 128:(ut + 1) * 128], attn,
                             start=True, stop=True)
            nc.vector.tensor_copy(oRT[:, ut], pr)

        # ---- proj + LN + residual -> OUT_L / OUT_R (natural [c, u]) ----
        def ln_block(src_ps, bias_t, gamma, beta, resid, out_tile, stag):
            if bias_t is not None:
                nc.vector.tensor_tensor(out=out_tile, in0=src_ps, in1=bias_t,
                                        op=OP.add)
            else:
                nc.vector.tensor_copy(out_tile, src_ps)
            stats = sm.tile([C, 6], f32, tag="stats" + stag)
            nc.vector.bn_stats(out=stats, in_=out_tile)
            mv = sm.tile([C, 2], f32, tag="mv" + stag)
            nc.vector.bn_aggr(out=mv, in_=stats)
            rstd = sm.tile([C, 1], f32, tag="rstd" + stag)
            nc.scalar.activation(out=rstd, in_=mv[:, 1:2], func=AF.Sqrt,
                                 bias=eps_t)
            nc.vector.reciprocal(rstd, rstd)
            nc.vector.tensor_scalar(out=out_tile, in0=out_tile,
                                    scalar1=mv[:, 0:1], scalar2=rstd,
                                    op0=OP.subtract, op1=OP.mult)
            nc.vector.tensor_tensor(out=out_tile, in0=out_tile, in1=gamma,
                                    op=OP.mult)
            nc.vector.tensor_tensor(out=out_tile, in0=out_tile, in1=beta,
                                    op=OP.add)
            nc.vector.tensor_tensor(out=out_tile, in0=out_tile, in1=resid,
                                    op=OP.add)

        OUT_L = wk.tile([C, T], f32, tag="OUTL")
        OUT_R = wk.tile([C, T], f32, tag="OUTR")
        for (oT, g, bt, resid, out_t) in [
                (oLT, vec["agl"], vec["abl"], xl_sb, OUT_L),
                (oRT, vec["agr"], vec["abr"], xr_sb, OUT_R)]:
            pp = ps.tile([C, T], f32, tag="ps")
            for ut in range(KT):
                nc.tensor.matmul(pp, oT[:, ut], wpt[:, ut],
                                 start=(ut == 0), stop=(ut == KT - 1))
            ln_block(pp, vec["bp"], g, bt, resid, out_t, "1")

        # ---- transposes of OUT_L/OUT_R -> [u(128), kt, c] tiles ----
        oLT2 = wk.tile([128, KT, C], f32, tag="oLT2")
        oRT2 = wk.tile([128, KT, C], f32, tag="oRT2")
        for (src, dst) in [(OUT_L, oLT2), (OUT_R, oRT2)]:
            for ut in range(KT):
                pt = ps.tile([128, C], f32, tag="ps")
                nc.tensor.transpose(pt, src[:, ut * 128:(ut + 1) * 128],
                                    ident[0:C, 0:C])
                nc.vector.tensor_copy(dst[:, ut], pt)

        # ---- router ----
        rtiles = [(oLT2, 0), (oLT2, 1), (oRT2, 0), (oRT2, 1)]
        # xp.T [d, c] (for the sim matmul's contraction over d)
        pxp = ps.tile([EXP, C], f32, tag="ps")
        for j, (tl, kt) in enumerate(rtiles):
            nc.tensor.matmul(pxp, wrt[:, j], tl[:, kt],
                             start=(j == 0), stop=(j == 3))
        xpT = wk.tile([EXP, C], f32, tag="xpT")
        nc.vector.tensor_scalar(out=xpT, in0=pxp, scalar1=brp, scalar2=None,
                                op0=OP.add)
        # xp natural [c, d] via PE transpose of xpT (for the row norms)
        pxn = ps.tile([C, EXP], f32, tag="ps")
        nc.tensor.transpose(pxn, xpT, ident[0:EXP, 0:EXP])
        xpn = sm.tile([C, EXP], f32, tag="xpn")
        nc.vector.tensor_copy(xpn, pxn)

        sq = sm.tile([C, EXP], f32, tag="sq")
        nc.vector.tensor_mul(sq, xpn, xpn)
        ssum = sm.tile([C, 1], f32, tag="ssum")
        nc.vector.tensor_reduce(ssum, sq, axis=mybir.AxisListType.X, op=OP.add)
        nc.scalar.activation(out=ssum, in_=ssum, func=AF.Sqrt)
        nc.vector.tensor_scalar_max(ssum, ssum, 1e-12)
        nc.vector.reciprocal(ssum, ssum)

        psim = ps.tile([C, C], f32, tag="ps")
        nc.tensor.matmul(psim, xpT, cent, start=True, stop=True)
        sim_sb = wk.tile([C, C], f32, tag="sim")
        nc.vector.tensor_scalar_mul(sim_sb, psim, ssum)

        mx8 = sm.tile([C, 8], f32, tag="mx8")
        nc.vector.max(out=mx8, in_=sim_sb)
        idx8 = sm.tile([C, 8], mybir.dt.uint32, tag="idx8")
        nc.vector.max_index(out=idx8, in_max=mx8, in_values=sim_sb)
        topif = sm.tile([C, 2], f32, tag="topif")
        nc.vector.tensor_copy(topif, idx8[:, 0:2])

        # ---- replicate topi rows across all 128 partitions via PE ----
        ptt = ps.tile([2, C], f32, tag="ps")
        nc.tensor.transpose(ptt, topif, ident[0:C, 0:C])
        ttT = sm.tile([2, C], f32, tag="ttT")
        nc.vector.tensor_copy(ttT, ptt)
        ttrep_ps = []
        for k in range(2):
            pr = ps.tile([128, C], f32, tag="ps")
            nc.tensor.matmul(pr, sel[:, k], ttT, start=True, stop=True)
            ttrep_ps.append(pr)

        # R.T[e, c] for the bias matmul (fp16)
        RT = wk.tile([C, C], f32, tag="RT")
        RT1 = sm.tile([C, C], f32, tag="RT1")
        nc.vector.tensor_scalar(out=RT, in0=ttrep_ps[0][0:C], scalar1=eiota,
                                scalar2=None, op0=OP.is_equal)
        nc.vector.tensor_scalar(out=RT1, in0=ttrep_ps[1][0:C], scalar1=eiota,
                                scalar2=None, op0=OP.is_equal)
        nc.vector.tensor_add(RT, RT, RT1)
        RTh = wk.tile([C, C], f16, tag="RTh")
        nc.vector.tensor_copy(RTh, RT)

        # ---- fp16 copy of transposed activations [p, kt, side, c] ----
        oAll = wk.tile([128, KT, 2, C], f16, tag="oAll")
        for kt in range(KT):
            nc.vector.tensor_copy(oAll[:, kt, 0], oLT2[:, kt])
            nc.vector.tensor_copy(oAll[:, kt, 1], oRT2[:, kt])

        # ---- ttrep rows as fp16 for the per-expert masks ----
        tt0r = wk.tile([128, C], f16, tag="tt0r")
        tt1r = wk.tile([128, C], f16, tag="tt1r")
        nc.vector.tensor_copy(tt0r, ttrep_ps[0])
        nc.vector.tensor_copy(tt1r, ttrep_ps[1])

        # ---- expert stage: per-expert mask + one fused multiply ----
        ps_moe = ps_moe_p.tile([128, T], f32, tag="psmoe")
        for e in range(C):
            m0 = msk_p.tile([128, C], f16, tag="m0")
            m1 = msk_p.tile([128, C], f16, tag="m1")
            nc.vector.tensor_scalar(out=m0, in0=tt0r, scalar1=float(e),
                                    scalar2=None, op0=OP.is_equal)
            nc.vector.tensor_scalar(out=m1, in0=tt1r, scalar1=float(e),
                                    scalar2=None, op0=OP.is_equal)
            nc.vector.tensor_add(m0, m0, m1)
            asc = asc_p.tile([128, KT, 2, C], f16, tag="asc")
            m0b = bass.AP(tensor=m0.tensor, offset=m0.offset,
                          ap=[list(m0.ap[0]), [0, KT], [0, 2], list(m0.ap[1])])
            nc.vector.tensor_tensor(out=asc, in0=oAll, in1=m0b, op=OP.mult)
            for kt in range(KT):
                nc.tensor.matmul(ps_moe, asc[:, kt], we_sb[:, e, kt],
                                 start=(e == 0 and kt == 0), stop=False,
                                 skip_group_check=True)
        nc.tensor.matmul(ps_moe[0:C], RTh, beh, start=False, stop=False,
                         skip_group_check=True)
        nc.tensor.matmul(ps_moe[C:128], RTh, beh, start=False, stop=True,
                         skip_group_check=True)

        # ---- final LN + residual ----
        ol2 = wk.tile([C, T], f32, tag="ol2")
        or2 = wk.tile([C, T], f32, tag="or2")
        ln_block(ps_moe[0:C], None, vec["mgl"], vec["mbl"], OUT_L, ol2, "2")
        ln_block(ps_moe[C:128], None, vec["mgr"], vec["mbr"], OUT_R, or2, "2")
        nc.sync.dma_start(out=ol2_d.ap(), in_=ol2)
        nc.sync.dma_start(out=or2_d.ap(), in_=or2)

    nc.compile()
    return nc


def _tile_t(w):
    # (T_in, N) -> [128, T_in//128, N] partition-tiled
    t_in, n = w.shape
    return np.ascontiguousarray(w.reshape(t_in // 128, 128, n).transpose(1, 0, 2))


def _prep_in_maps(inputs):
    f = np.float32
    x_l, x_r = inputs["x_l"], inputs["x_r"]

    def rep(name):
        return np.repeat(np.asarray(inputs[name], f).reshape(1, T), C, axis=0)

    cen = np.asarray(inputs["centers"], f)
    cenn = cen / np.maximum(np.linalg.norm(cen, axis=-1, keepdims=True), 1e-12)
    sel = np.zeros((2, 2, 128), f)
    sel[0, 0, :] = 1.0
    sel[1, 1, :] = 1.0
    arrs = {
        "wqt": _tile_t(np.asarray(inputs["Wq"], f).T),
        "wkt": _tile_t(np.asarray(inputs["Wk"], f).T),
        "wvt": _tile_t(np.asarray(inputs["Wv"], f).T),
        "wpt": _tile_t(np.asarray(inputs["Wp"], f).T),
        "bqp": np.asarray(inputs["bq"], f).reshape(KT, 128).T,
        "bkp": np.asarray(inputs["bk"], f).reshape(KT, 128).T,
        "wrt": _tile_t(np.asarray(inputs["Wr"], f).T),
        "brp": np.asarray(inputs["br"], f).reshape(EXP, 1),
        "cent": np.ascontiguousarray(cenn.T),
        "ident": np.eye(128, dtype=f),
        "eiota": np.arange(C, dtype=f).reshape(C, 1),
        "sel": sel,
        "bv": rep("bv"), "bp": rep("bp"),
        "agl": rep("ag_l"), "abl": rep("ab_l"),
        "agr": rep("ag_r"), "abr": rep("ab_r"),
        "mgl": rep("mg_l"), "mbl": rep("mb_l"),
        "mgr": rep("mg_r"), "mbr": rep("mb_r"),
        "xl": np.zeros((C, T), f), "xr": np.zeros((C, T), f),
    }
    We = np.asarray(inputs["We"], f)
    WeTh = np.ascontiguousarray(
        We.transpose(0, 2, 1).reshape(C, KT, 128, T).transpose(2, 0, 1, 3)
    ).astype(np.float16)
    beh = np.asarray(inputs["be"], f).astype(np.float16)

    def pack(spec, ncols, extra):
        blob = np.zeros((128, ncols), f)
        for name, parts, shape in spec:
            off, _, _ = BLOB_OFF[name]
            cols = int(np.prod(shape[1:]))
            a = extra[name] if name in extra else arrs[name]
            blob[0:parts, off:off + cols] = np.asarray(a, f).reshape(parts, cols)
        return blob

    blobB = pack(BLOB_B_SPEC, NB_COLS, {})
    in_maps = []
    for b in range(N_CORES):
        xtl = _tile_t(np.ascontiguousarray(np.asarray(x_l[b], f).T))
        xtr = _tile_t(np.ascontiguousarray(np.asarray(x_r[b], f).T))
        blobA = pack(BLOB_A_SPEC, NA_COLS, {"xtl": xtl, "xtr": xtr})
        bB = blobB.copy()
        o, p, sh = BLOB_OFF["xl"]
        bB[0:p, o:o + T] = np.asarray(x_l[b], f)
        o, p, sh = BLOB_OFF["xr"]
        bB[0:p, o:o + T] = np.asarray(x_r[b], f)
        in_maps.append({"blobA": blobA, "blobB": bB, "weh": WeTh, "beh": beh})
    return in_maps


def kernel(**inputs) -> np.ndarray:
    from concourse.bass_utils import run_bass_kernel_spmd

    if "nc" not in _CACHE:
        _CACHE["nc"] = _build()
    nc = _CACHE["nc"]
    in_maps = _prep_in_maps(inputs)
    res = run_bass_kernel_spmd(nc, in_maps, list(range(N_CORES)))
    _CACHE["exec_time_ns"] = res.exec_time_ns
    out_l2 = np.stack([res.results[b]["ol2"] for b in range(N_CORES)])
    out_r2 = np.stack([res.results[b]["or2"] for b in range(N_CORES)])
    return np.stack([out_l2, out_r2]).astype(np.float32)
